# revision 21
# baseline (speedup 1.0000x reference)
"""BiLSTM-CRF Trainium2 kernel (8-core SPMD, batch-sharded), v2.

Per core: 4 sequences, full pipeline on device:
  embedding gather (indirect DMA) -> PE transposes -> input-gate GEMMs ->
  512-step bidirectional LSTM recurrence -> emission GEMM ->
  chunked-parallel Viterbi scan (K=9 live states) -> batched pointer
  extraction -> composed-pointer backtrace -> int32 tag path.

v2 changes vs v1:
  - Viterbi runs on the 9 live states only (START/STOP/PAD provably never
    win for t>=1; t=0 handled by a constant-column operator A_0 with a
    zero initial score vector).
  - The 512-step serial scan is replaced by a chunked parallel scan:
    32 chunks x 16 steps laid out [(chunk,b) partition, (step,j,k) free],
    in-chunk max-plus operator trees (batched over 128 partitions),
    a 32-step serial scan over chunk boundaries only, then a 16-step
    batched interior reconstruction.
  - Pointer extraction is one batched masked-argmax over all (t, j).
  - Backtrace hops 8 steps per serial iteration via composed pointer maps
    (ptr2/ptr4/ptr8); the 7 interior tags per anchor are reconstructed in
    7 batched rounds afterwards.

Math notes:
  sigmoid(x) = 0.5*tanh(0.5x)+0.5 so every gate uses one Tanh activation; the
  0.5 factors are pre-folded into the weights. Cell/hidden state are carried
  doubled (C=2c, H=2h) so the whole cell update is 4 fused
  scalar_tensor_tensor ops; the 0.5 for H is folded into W_hh and W_out.
"""

import numpy as np

import concourse.bass as bass
import concourse.tile as tile
from concourse import bacc, mybir
from concourse.bass_utils import run_bass_kernel_spmd

FP = mybir.dt.float32
I32 = mybir.dt.int32
AX = mybir.AxisListType
OP = mybir.AluOpType
AF = mybir.ActivationFunctionType

VOCAB = 100000
E = 256
Hh = 128
K = 12
K9 = 9
START = 9
STOP = 10
NEG = -10000.0
B = 32
NCORES = 8
BL = B // NCORES  # 4 sequences per core

# Viterbi chunking: 32 chunks x 16 steps (chunk c, in-chunk step s; t=16c+s)
NCH = 32
CL = 16


def build_program(T=512, dbg=False):
    nc = bacc.Bacc("TRN2", target_bir_lowering=False, debug=False)
    NTOK = T * BL              # tokens per core
    NTILE = NTOK // 128        # gather / feat tiles (16 at T=512)
    NCHUNK = NTOK // 512       # 512-col GEMM chunks (4)
    NANCH = T // 8             # backtrace anchors (64)

    def din(name, shape, dtype=FP):
        return nc.dram_tensor(name, list(shape), dtype, kind="ExternalInput").ap()

    idx_in = din("idx_in", [128, NTILE], I32)          # [p,k] token ids, time-major
    embed = din("embed", [VOCAB, E])
    w_ihT = din("w_ihT", [2, E, 4 * Hh])               # pre-scaled, gate order i,f,o,g
    w_hhT = din("w_hhT", [2, Hh, 4 * Hh])
    b_in = din("b_in", [128, 8])                       # col d*4+g: per-partition bias
    h_init = din("h_init", [2, 128, BL])               # 2*h0, feature-major
    c_init = din("c_init", [2, 128, BL])               # 2*c0
    w_outT = din("w_outT", [2, Hh, K9])                # 0.5*W_out halves, transposed, 9 tags
    bout_rep = din("bout_rep", [128, K9])
    ident = din("ident", [128, 128])
    tr9 = din("tr9", [128, K9 * K9])                   # trans[j,k] j,k in 0..8, replicated
    tstart9 = din("tstart9", [128, K9])                # trans[j,START] replicated
    tstop9 = din("tstop9", [BL, K9])                   # trans[STOP,0:9] replicated
    wvec9 = din("wvec9", [128, K9])                    # 8-k, replicated
    ivec9 = din("ivec9", [128, K9])                    # k, replicated

    path_out = nc.dram_tensor("path_out", [BL, T], I32, kind="ExternalOutput").ap()

    # DRAM scratch for partition-permute bounces
    ffd = nc.dram_tensor("ffd", [NCH, BL, CL, K9], FP).ap()       # feats (c,b,s,j)
    m_d = nc.dram_tensor("m_d", [NCH, BL, K9 * K9], FP).ap()      # chunk operators
    sb_d = nc.dram_tensor("sb_d", [NCH, BL, K9], FP).ap()         # chunk boundary scores
    w_d = nc.dram_tensor("w_d", [NCH, BL, CL * K9], FP).ap()      # single-step pointers
    w8_d = nc.dram_tensor("w8_d", [NCH, BL, CL * K9], FP).ap()    # 8-step composed maps

    # chunk emission orders: fwd consumes chunks 0..3, rev consumes 3..0
    CHUNK_ORDER = [(0, 0), (1, NCHUNK - 1)]
    for i in range(1, NCHUNK):
        CHUNK_ORDER.append((0, i))
        CHUNK_ORDER.append((1, NCHUNK - 1 - i))
    TILE_ORDER = []
    seen = set()
    for d, c in CHUNK_ORDER:
        for k in range(c * 4, c * 4 + 4):
            if k not in seen:
                seen.add(k)
                TILE_ORDER.append(k)

    with tile.TileContext(nc) as tc:
        with tc.tile_pool(name="const", bufs=1) as cpool, \
             tc.tile_pool(name="big", bufs=1) as bpool:

            # ---- load constants ----
            def cload(ap_in, shape, dtype=FP):
                t = cpool.tile(list(shape), dtype, name=f"c_{np.random.randint(1<<30)}")
                nc.sync.dma_start(t[:], ap_in)
                return t

            idx_sb = cload(idx_in, [128, NTILE], I32)
            wih_sb = [[cload(w_ihT[d, e * 128:(e + 1) * 128, :], [128, 4 * Hh])
                       for e in range(2)] for d in range(2)]
            whh_sb = [cload(w_hhT[d], [Hh, 4 * Hh]) for d in range(2)]
            b_sb = cload(b_in, [128, 8])
            hi_sb = [cload(h_init[d], [128, BL]) for d in range(2)]
            ci_sb = [cload(c_init[d], [128, BL]) for d in range(2)]
            wout_sb = [cload(w_outT[d], [Hh, K9]) for d in range(2)]
            bout_sb = cload(bout_rep, [128, K9])
            id_sb = cload(ident, [128, 128])
            tr9_sb = cload(tr9, [128, K9 * K9])
            tst9_sb = cload(tstart9, [128, K9])
            tsp9_sb = cload(tstop9, [BL, K9])
            wv9_sb = cload(wvec9, [128, K9])
            iv9_sb = cload(ivec9, [128, K9])

            # big persistent arrays
            xg_sb = [bpool.tile([128, T * 16], FP, tag=f"xg{d}", name=f"xg{d}") for d in range(2)]
            hs_sb = [bpool.tile([128, T * BL], FP, tag=f"hs{d}", name=f"hs{d}") for d in range(2)]
            ft9 = bpool.tile([128, NTILE * K9], FP, tag="ft9", name="ft9")

            # ---- phase 1: embedding gather + transpose to [E, tok] ----
            from contextlib import ExitStack as _ES
            _xe_es = _ES()
            xepool = _xe_es.enter_context(tc.tile_pool(name="xe", bufs=1))
            xe_sb = [xepool.tile([128, NTOK], FP, tag=f"xe{e}", name=f"xe{e}") for e in range(2)]
            with tc.tile_pool(name="gat", bufs=3) as gpool, \
                 tc.tile_pool(name="ps1", bufs=4, space="PSUM") as ps1:
                for k in TILE_ORDER:
                    gt = gpool.tile([128, E], FP)
                    nc.gpsimd.indirect_dma_start(
                        out=gt[:],
                        out_offset=None,
                        in_=embed[:],
                        in_offset=bass.IndirectOffsetOnAxis(
                            ap=idx_sb[:, k:k + 1], axis=0),
                    )
                    for e in range(2):
                        pt = ps1.tile([128, 128], FP, space="PSUM")
                        nc.tensor.transpose(
                            out=pt[:], in_=gt[:, e * 128:(e + 1) * 128],
                            identity=id_sb[:])
                        nc.vector.tensor_copy(
                            xe_sb[e][:, k * 128:(k + 1) * 128], pt[:])

            # ---- phase 2: xg = W_ih_eff @ xe + b, interleaved [t,(g,b)] ----
            # Only the first two chunks (fwd t=0.., rev t=T-1..) run before the
            # recurrence; the remaining 6 chunks are drip-fed into the step
            # loop (one e-pass per step) so they execute in PE idle windows
            # instead of serializing ahead of the recurrence in the PE FIFO.
            _ps2_ref = [None]

            def emit_gemm_unit(d, c, g, e, pt_live):
                xgv = xg_sb[d][:].rearrange("p (t x) -> p t x", x=16)
                if e == 0:
                    pt = _ps2_ref[0].tile([128, 512], FP, space="PSUM")
                    pt_live[(d, c, g)] = pt
                else:
                    pt = pt_live.pop((d, c, g))
                nc.tensor.matmul(
                    pt[:],
                    lhsT=wih_sb[d][e][:, g * 128:(g + 1) * 128],
                    rhs=xe_sb[e][:, c * 512:(c + 1) * 512],
                    start=(e == 0), stop=(e == 1),
                )
                if e == 1:
                    nc.vector.tensor_scalar(
                        out=xgv[:, c * 128:(c + 1) * 128,
                                g * 4:(g + 1) * 4],
                        in0=pt[:].rearrange("p (t b) -> p t b", b=BL),
                        scalar1=b_sb[:, d * 4 + g:d * 4 + g + 1],
                        scalar2=None,
                        op0=OP.add,
                    )

            _gemm_units = [(d, c, g, e)
                           for d, c in CHUNK_ORDER[2:]
                           for g in range(4)
                           for e in range(2)]

            # ---- phase 3: LSTM recurrence, dir-major so the scheduler can
            # pipeline one direction's PE block against the other's chain ----
            # gate cols per step: i=0:4, f=4:8, o=8:12, g=12:16
            # phase 4 pools opened alongside so emission GEMMs can overlap the
            # recurrence tail.
            with tc.tile_pool(name="ps3", bufs=3, space="PSUM") as ps3, \
                 tc.tile_pool(name="th", bufs=4) as thpool, \
                 tc.tile_pool(name="cell", bufs=4) as cellpool, \
                 tc.tile_pool(name="cst", bufs=2) as cstpool:
                _ps2_es = _ES()
                _ps2_ref[0] = _ps2_es.enter_context(
                    tc.tile_pool(name="ps2", bufs=2, space="PSUM"))
                _pt_live = {}
                for d, c in CHUNK_ORDER[:2]:
                    for g in range(4):
                        for e in range(2):
                            emit_gemm_unit(d, c, g, e, _pt_live)
                _ps4_es = _ES()
                ps4 = None
                c_cur = [ci_sb[0], ci_sb[1]]
                for step in range(T):
                    if _gemm_units and step >= 4 and step % 3 == 0:
                        emit_gemm_unit(*_gemm_units.pop(0), _pt_live)
                        if not _gemm_units:
                            _ps2_es.close()
                            ps4 = _ps4_es.enter_context(
                                tc.tile_pool(name="ps4", bufs=2,
                                             space="PSUM"))
                    tt = [step, T - 1 - step]
                    for d in range(2):
                        t_d = tt[d]
                        prev = (hi_sb[d][:] if step == 0 else
                                hs_sb[d][:, (t_d - 1 + 2 * d) * BL:
                                          (t_d + 2 * d) * BL])
                        p = ps3.tile([128, 16], FP, space="PSUM",
                                     tag=f"g{d}", name=f"g{d}_{step}")
                        for q in range(4):
                            nc.tensor.matmul(
                                p[32 * q:32 * (q + 1), :],
                                lhsT=id_sb[:, 32 * q:32 * (q + 1)],
                                rhs=xg_sb[d][:, t_d * 16:(t_d + 1) * 16],
                                start=True, stop=False,
                                tile_position=(0, 32 * q),
                                skip_group_check=True)
                        for g in range(4):
                            for q in range(4):
                                nc.tensor.matmul(
                                    p[32 * q:32 * (q + 1), g * 4:(g + 1) * 4],
                                    lhsT=whh_sb[d][:, g * 128 + 32 * q:
                                                   g * 128 + 32 * (q + 1)],
                                    rhs=prev,
                                    start=False, stop=(g == 3 and q == 3),
                                    tile_position=(0, 32 * q),
                                    skip_group_check=True)
                        th = thpool.tile([128, 16], FP, tag=f"th{d}",
                                         name=f"th{d}_{step}")
                        nc.scalar.activation(th[:], p[:], AF.Tanh)
                        a_t = cellpool.tile([128, BL], FP, tag=f"a{d}",
                                            name=f"a{d}_{step}")
                        b_t = cellpool.tile([128, BL], FP, tag=f"b{d}",
                                            name=f"b{d}_{step}")
                        nc.vector.scalar_tensor_tensor(
                            out=a_t[:], in0=th[:, 4:8], scalar=1.0,
                            in1=c_cur[d][:], op0=OP.add, op1=OP.mult)
                        nc.vector.scalar_tensor_tensor(
                            out=b_t[:], in0=th[:, 0:4], scalar=1.0,
                            in1=th[:, 12:16], op0=OP.add, op1=OP.mult)
                        c_n = cstpool.tile([128, BL], FP, tag=f"c{d}",
                                           name=f"c{d}_{step}")
                        nc.vector.scalar_tensor_tensor(
                            out=c_n[:], in0=a_t[:], scalar=0.5,
                            in1=b_t[:], op0=OP.mult, op1=OP.add)
                        tc_t = cellpool.tile([128, BL], FP, tag=f"tc{d}",
                                             name=f"tc{d}_{step}")
                        nc.scalar.activation(tc_t[:], c_n[:], AF.Tanh,
                                             scale=0.5)
                        nc.vector.scalar_tensor_tensor(
                            out=hs_sb[d][:, t_d * BL:(t_d + 1) * BL],
                            in0=th[:, 8:12], scalar=1.0,
                            in1=tc_t[:], op0=OP.add, op1=OP.mult)
                        c_cur[d] = c_n

                # ---- phase 4: emission scores -> ft9 [p,(c2,j)] + DRAM bounce
                # to (chunk,b)-major. Tile c2 covers chunks 2*c2, 2*c2+1.
                for ch in range(NTILE):
                    pt = ps4.tile([128, K9], FP, space="PSUM")
                    for d in range(2):
                        nc.tensor.matmul(
                            pt[:],
                            lhsT=hs_sb[d][:, ch * 128:(ch + 1) * 128],
                            rhs=wout_sb[d][:],
                            start=(d == 0), stop=(d == 1))
                    nc.vector.tensor_add(
                        ft9[:, ch * K9:(ch + 1) * K9], pt[:], bout_sb[:])
                    for c0 in range(2):
                        nc.sync.dma_start(
                            ffd[2 * ch + c0].rearrange("b s j -> s b j"),
                            ft9[64 * c0:64 * (c0 + 1),
                                ch * K9:(ch + 1) * K9])
                _ps4_es.close()

            # ---- phase 5: Viterbi chunked parallel scan (K=9) ----
            with tc.tile_pool(name="vit", bufs=1) as vp, \
                 tc.tile_pool(name="vtmp", bufs=2) as vt:
                # f_cb[(c,b), (s,j)] feats in chunk-major layout
                f_cb = vp.tile([128, CL * K9], FP, name="f_cb")
                nc.sync.dma_start(
                    f_cb[:], ffd.rearrange("c b s j -> (c b) (s j)"))
                # A_cb[(c,b), (s,j,k)] = T9[j,k] + feat[t=16c+s, b, j]
                A_cb = vp.tile([128, CL * K9 * K9], FP, name="A_cb")
                A4 = A_cb[:].rearrange("p (s j k) -> p s j k", j=K9, k=K9)
                nc.vector.tensor_tensor(
                    out=A4,
                    in0=f_cb[:].rearrange("p (s j) -> p s j", j=K9)
                        .unsqueeze(3).broadcast_to([128, CL, K9, K9]),
                    in1=tr9_sb[:].rearrange("p (j k) -> p j k", k=K9)
                        .unsqueeze(1).broadcast_to([128, CL, K9, K9]),
                    op=OP.add)
                # t=0 fix: A_0[j,k] = feat_0[j] + trans[j,START] (k-const),
                # paired with a zero initial score vector.
                nc.vector.tensor_tensor(
                    out=A_cb[0:BL, 0:K9 * K9].rearrange(
                        "p (j k) -> p j k", k=K9),
                    in0=f_cb[0:BL, 0:K9].unsqueeze(2)
                        .broadcast_to([BL, K9, K9]),
                    in1=tst9_sb[0:BL, :].unsqueeze(2)
                        .broadcast_to([BL, K9, K9]),
                    op=OP.add)
                # A_T[p, s, m, k] = A[s, k, m] (transposed view for compose)
                A_T = A_cb[:].rearrange("p (s a b) -> p s b a", a=K9, b=K9)

                # in-chunk operator trees: compose pairs (later o earlier)
                # C[j,m] = max_k L[j,k] + E[k,m]
                def compose_ops(out_flat, L_ap, E_T_ap):
                    # out_flat: [p, 81]; L_ap: [p, j, k]; E_T_ap: [p, m, k]
                    tmp = vt.tile([128, K9 * K9 * K9], FP, tag="ctmp")
                    nc.vector.tensor_tensor(
                        out=tmp[:].rearrange("p (j m k) -> p j m k",
                                             m=K9, k=K9),
                        in0=L_ap.unsqueeze(2).broadcast_to(
                            [128, K9, K9, K9]),
                        in1=E_T_ap.unsqueeze(1).broadcast_to(
                            [128, K9, K9, K9]),
                        op=OP.add)
                    nc.vector.reduce_max(
                        out_flat,
                        tmp[:].rearrange("p (x k) -> p x k", k=K9),
                        axis=AX.X)

                M2 = vp.tile([128, 8 * K9 * K9], FP, name="M2")
                for u in range(8):
                    compose_ops(
                        M2[:, u * 81:(u + 1) * 81],
                        A4[:, 2 * u + 1], A_T[:, 2 * u])
                M2v = M2[:].rearrange("p (u j m) -> p u j m", j=K9, m=K9)
                M2T = M2[:].rearrange("p (u a b) -> p u b a", a=K9, b=K9)
                M4 = vp.tile([128, 4 * K9 * K9], FP, name="M4")
                for u in range(4):
                    compose_ops(
                        M4[:, u * 81:(u + 1) * 81],
                        M2v[:, 2 * u + 1], M2T[:, 2 * u])
                M4v = M4[:].rearrange("p (u j m) -> p u j m", j=K9, m=K9)
                M4T = M4[:].rearrange("p (u a b) -> p u b a", a=K9, b=K9)
                M8 = vp.tile([128, 2 * K9 * K9], FP, name="M8")
                for u in range(2):
                    compose_ops(
                        M8[:, u * 81:(u + 1) * 81],
                        M4v[:, 2 * u + 1], M4T[:, 2 * u])
                Mc = vp.tile([128, K9 * K9], FP, name="Mc")
                compose_ops(
                    Mc[:],
                    M8[:].rearrange("p (u j m) -> p u j m", j=K9, m=K9)[:, 1],
                    M8[:].rearrange("p (u a b) -> p u b a", a=K9, b=K9)[:, 0])

                # bounce chunk operators to b-major for the serial scan
                nc.sync.dma_start(m_d.rearrange("c b jk -> (c b) jk"), Mc[:])
                Mb4 = vp.tile([BL, NCH * K9 * K9], FP, name="Mb4")
                nc.sync.dma_start(
                    Mb4[:].rearrange("b (c jk) -> b c jk", jk=K9 * K9),
                    m_d.rearrange("c b jk -> b c jk"))

                # serial scan over the 32 chunk boundaries; B[:, c*9:] = B_c
                Bt = vp.tile([BL, (NCH + 1) * K9], FP, name="Bt")
                nc.vector.memset(Bt[:, 0:K9], 0.0)
                Mb4v = Mb4[:].rearrange("b (c j k) -> b c j k", j=K9, k=K9)
                for c in range(NCH):
                    btmp = vt.tile([BL, K9 * K9], FP, tag="btmp")
                    nc.vector.tensor_tensor(
                        out=btmp[:].rearrange("b (j k) -> b j k", k=K9),
                        in0=Bt[:, c * K9:(c + 1) * K9].unsqueeze(1)
                            .broadcast_to([BL, K9, K9]),
                        in1=Mb4v[:, c], op=OP.add)
                    nc.vector.reduce_max(
                        Bt[:, (c + 1) * K9:(c + 2) * K9],
                        btmp[:].rearrange("b (j k) -> b j k", k=K9),
                        axis=AX.X)

                # bounce boundaries back to (c,b)-major
                nc.sync.dma_start(
                    sb_d.rearrange("c b j -> b c j"),
                    Bt[:, 0:NCH * K9])
                B_cb = vp.tile([128, K9], FP, name="B_cb")
                nc.sync.dma_start(B_cb[:], sb_d.rearrange("c b j -> (c b) j"))

                # interior reconstruction: 16 serial steps over all chunks
                S_store = vp.tile([128, CL * K9], FP, name="S_store")
                for s in range(CL):
                    sprev = (B_cb[:] if s == 0 else
                             S_store[:, (s - 1) * K9:s * K9])
                    stmp = vt.tile([128, K9 * K9], FP, tag="stmp")
                    nc.vector.tensor_tensor(
                        out=stmp[:].rearrange("p (j k) -> p j k", k=K9),
                        in0=sprev.unsqueeze(1).broadcast_to([128, K9, K9]),
                        in1=A4[:, s], op=OP.add)
                    nc.vector.reduce_max(
                        S_store[:, s * K9:(s + 1) * K9],
                        stmp[:].rearrange("p (j k) -> p j k", k=K9),
                        axis=AX.X)

                # ---- phase 6: batched pointer extraction ----
                # Sprev[(c,b), s, k] = scores before step t=16c+s
                Sprev = vp.tile([128, CL * K9], FP, name="Sprev")
                nc.vector.tensor_copy(Sprev[:, 0:K9], B_cb[:])
                nc.vector.tensor_copy(Sprev[:, K9:], S_store[:, 0:(CL - 1) * K9])
                mbig = vp.tile([128, CL * K9 * K9], FP, name="mbig")
                mb4 = mbig[:].rearrange("p (s j k) -> p s j k", j=K9, k=K9)
                nc.vector.tensor_tensor(
                    out=mb4,
                    in0=Sprev[:].rearrange("p (s k) -> p s k", k=K9)
                        .unsqueeze(2).broadcast_to([128, CL, K9, K9]),
                    in1=tr9_sb[:].rearrange("p (j k) -> p j k", k=K9)
                        .unsqueeze(1).broadcast_to([128, CL, K9, K9]),
                    op=OP.add)
                mx = vp.tile([128, CL * K9], FP, name="mx")
                nc.vector.reduce_max(
                    mx[:], mbig[:].rearrange("p (x k) -> p x k", k=K9),
                    axis=AX.X)
                nc.vector.tensor_tensor(
                    out=mb4, in0=mb4,
                    in1=mx[:].rearrange("p (s j) -> p s j", j=K9)
                        .unsqueeze(3).broadcast_to([128, CL, K9, K9]),
                    op=OP.is_equal)
                nc.vector.tensor_tensor(
                    out=mb4, in0=mb4,
                    in1=wv9_sb[:].unsqueeze(1).unsqueeze(1)
                        .broadcast_to([128, CL, K9, K9]),
                    op=OP.mult)
                ptr = vp.tile([128, CL * K9], FP, name="ptr")
                nc.vector.reduce_max(
                    ptr[:], mbig[:].rearrange("p (x k) -> p x k", k=K9),
                    axis=AX.X)
                # codes -> indices: ptr = 8 - code
                nc.vector.tensor_scalar(
                    out=ptr[:], in0=ptr[:], scalar1=-1.0,
                    scalar2=float(K9 - 1), op0=OP.mult, op1=OP.add)

                # ---- composed pointer maps ptr2 / ptr4 / ptr8 ----
                _shn = [0]

                def shift_map(src, nsteps):
                    _shn[0] += 1
                    sh = vp.tile([128, CL * K9], FP,
                                 name=f"shm_{_shn[0]}")
                    nc.vector.tensor_copy(
                        sh[:, nsteps * K9:], src[:, 0:(CL - nsteps) * K9])
                    nc.sync.dma_start(
                        sh[BL:128, 0:nsteps * K9],
                        src[0:128 - BL, (CL - nsteps) * K9:])
                    nc.vector.memset(sh[0:BL, 0:nsteps * K9], 0.0)
                    return sh

                def compose_maps(cur, sh, name):
                    # new[x] = sh[cur[x]]
                    out = vp.tile([128, CL * K9], FP, name=name)
                    e3 = vt.tile([128, CL * K9 * K9], FP, tag="e3")
                    e3v = e3[:].rearrange("p (s x m) -> p s x m", x=K9, m=K9)
                    nc.vector.tensor_tensor(
                        out=e3v,
                        in0=cur[:].rearrange("p (s x) -> p s x", x=K9)
                            .unsqueeze(3).broadcast_to([128, CL, K9, K9]),
                        in1=iv9_sb[:].unsqueeze(1).unsqueeze(1)
                            .broadcast_to([128, CL, K9, K9]),
                        op=OP.is_equal)
                    nc.vector.tensor_tensor(
                        out=e3v, in0=e3v,
                        in1=sh[:].rearrange("p (s m) -> p s m", m=K9)
                            .unsqueeze(2).broadcast_to([128, CL, K9, K9]),
                        op=OP.mult)
                    nc.vector.reduce_max(
                        out[:], e3[:].rearrange("p (x m) -> p x m", m=K9),
                        axis=AX.X)
                    return out

                sh1 = shift_map(ptr, 1)
                ptr2 = compose_maps(ptr, sh1, "ptr2")
                sh2 = shift_map(ptr2, 2)
                ptr4 = compose_maps(ptr2, sh2, "ptr4")
                sh4 = shift_map(ptr4, 4)
                ptr8 = compose_maps(ptr4, sh4, "ptr8")

                # bounce ptr and ptr8 to b-major [BL, (t, j)]
                nc.sync.dma_start(
                    w_d.rearrange("c b sj -> (c b) sj"), ptr[:])
                nc.sync.dma_start(
                    w8_d.rearrange("c b sj -> (c b) sj"), ptr8[:])
                wptr4 = vp.tile([BL, T * K9], FP, name="wptr4")
                nc.sync.dma_start(
                    wptr4[:].rearrange("b (c sj) -> b c sj", sj=CL * K9),
                    w_d.rearrange("c b sj -> b c sj"))
                wptr8 = vp.tile([BL, T * K9], FP, name="wptr8")
                nc.sync.dma_start(
                    wptr8[:].rearrange("b (c sj) -> b c sj", sj=CL * K9),
                    w8_d.rearrange("c b sj -> b c sj"))

                # ---- phase 7: init best tag + backtrace ----
                wpath = vp.tile([BL, T], FP, name="wpath")
                fs0 = vp.tile([BL, K9], FP, name="fs0")
                nc.sync.dma_start(
                    fs0[:], S_store[128 - BL:128, (CL - 1) * K9:])
                fs = vp.tile([BL, K9], FP, name="fs")
                nc.vector.tensor_add(fs[:], fs0[:], tsp9_sb[:])
                mx8 = vp.tile([BL, 8], FP, name="mx8")
                nc.vector.max(mx8[:], fs[:])
                bmsk = vp.tile([BL, K9], FP, name="bmsk")
                nc.vector.tensor_scalar(
                    out=bmsk[:], in0=fs[:], scalar1=mx8[:, 0:1], scalar2=None,
                    op0=OP.is_equal)
                nc.vector.tensor_mul(bmsk[:], bmsk[:], wv9_sb[0:BL, :])
                cod = vp.tile([BL, 1], FP, name="cod")
                nc.vector.reduce_max(cod[:], bmsk[:], axis=AX.X)
                nc.vector.tensor_scalar(
                    out=wpath[:, T - 1:T], in0=cod[:], scalar1=-1.0,
                    scalar2=float(K9 - 1), op0=OP.mult, op1=OP.add)

                # serial anchor hops: 8 steps per iteration via ptr8
                oh = vp.tile([BL, K9], FP, name="oh")
                scr = vp.tile([BL, K9], FP, name="scr")
                nc.vector.tensor_scalar(
                    out=oh[:], in0=iv9_sb[0:BL, :],
                    scalar1=wpath[:, T - 1:T], scalar2=None, op0=OP.is_equal)
                for a in range(T - 1, 14, -8):
                    nc.vector.scalar_tensor_tensor(
                        out=scr[:], in0=oh[:], scalar=1.0,
                        in1=wptr8[:, a * K9:(a + 1) * K9],
                        op0=OP.mult, op1=OP.mult,
                        accum_out=wpath[:, a - 8:a - 7])
                    nc.vector.tensor_scalar(
                        out=oh[:], in0=iv9_sb[0:BL, :],
                        scalar1=wpath[:, a - 8:a - 7], scalar2=None,
                        op0=OP.is_equal)

                # batched interior rounds: tags at anchor-r, r=1..7
                wp8 = wpath[:].rearrange("b (k eight) -> b k eight", eight=8)
                wptr_t = wptr4[:].rearrange(
                    "b (k eight j) -> b k eight j", eight=8, j=K9)
                for r in range(1, 8):
                    ohall = vt.tile([BL, NANCH * K9], FP, tag="ohall")
                    oav = ohall[:].rearrange("b (a j) -> b a j", j=K9)
                    nc.vector.tensor_tensor(
                        out=oav,
                        in0=wp8[:, :, 8 - r].unsqueeze(2)
                            .broadcast_to([BL, NANCH, K9]),
                        in1=iv9_sb[0:BL, :].unsqueeze(1)
                            .broadcast_to([BL, NANCH, K9]),
                        op=OP.is_equal)
                    nc.vector.tensor_tensor(
                        out=oav, in0=oav,
                        in1=wptr_t[:, :, 8 - r], op=OP.mult)
                    nc.vector.reduce_max(
                        wp8[:, :, 7 - r], oav, axis=AX.X)

                # ---- phase 8: path -> int32 -> out ----
                pi = vp.tile([BL, T], I32, name="pi")
                nc.vector.tensor_scalar(
                    out=pi[:], in0=wpath[:], scalar1=1.0, scalar2=0.0,
                    op0=OP.mult, op1=OP.add)
                nc.sync.dma_start(path_out, pi[:])

                if dbg:
                    def dump(name, t, shape):
                        dt_ = nc.dram_tensor(name, list(shape), FP,
                                             kind="ExternalOutput").ap()
                        nc.sync.dma_start(dt_, t)
                    dump("d_fcb", f_cb[:], [128, CL * K9])
                    dump("d_Acb", A_cb[:], [128, CL * K9 * K9])
                    dump("d_Mc", Mc[:], [128, K9 * K9])
                    dump("d_Bt", Bt[:], [BL, (NCH + 1) * K9])
                    dump("d_S", S_store[:], [128, CL * K9])
                    dump("d_ptr", ptr[:], [128, CL * K9])
                    dump("d_ptr8", ptr8[:], [128, CL * K9])
                    dump("d_wptr4", wptr4[:], [BL, T * K9])
                    dump("d_wptr8", wptr8[:], [BL, T * K9])
                    dump("d_wpath", wpath[:], [BL, T])
                    dump("d_fs", fs[:], [BL, K9])

            _xe_es.close()
    nc.compile()
    return nc


def prep_inputs(sentence, h0, c0, embed, W_ih_f, W_hh_f, b_f, W_ih_r, W_hh_r,
                b_r, W_out, b_out, transitions, T=512):
    """Host-side layout prep. Returns per-core input maps."""
    f32 = np.float32
    perm = np.r_[0:128, 128:256, 384:512, 256:384]  # i,f,g,o -> i,f,o,g
    gs = np.concatenate([np.full(128, s, f32) for s in (0.5, 0.5, 0.5, 1.0)])

    def prep_dir(W_ih, W_hh, b):
        Wi = np.asarray(W_ih, f32)[perm] * gs[:, None]
        bb = np.asarray(b, f32)[perm] * gs
        Wh = np.asarray(W_hh, f32)[perm] * (0.5 * gs)[:, None]
        return Wi.T.copy(), Wh.T.copy(), bb

    wihT_f, whhT_f, be_f = prep_dir(W_ih_f, W_hh_f, b_f)
    wihT_r, whhT_r, be_r = prep_dir(W_ih_r, W_hh_r, b_r)
    w_ihT = np.stack([wihT_f, wihT_r])
    w_hhT = np.stack([whhT_f, whhT_r])
    b_in = np.stack([be_f.reshape(4, 128), be_r.reshape(4, 128)])  # [2,4,128]
    b_in = b_in.reshape(8, 128).T.copy()                           # [128,8]

    Wo = np.asarray(W_out, f32) * 0.5
    w_outT = np.stack([Wo[0:K9, :128].T.copy(), Wo[0:K9, 128:].T.copy()])
    bout_rep = np.tile(np.asarray(b_out, f32)[None, 0:K9], (128, 1))

    tr = np.asarray(transitions, f32)
    tr9 = np.tile(tr[0:K9, 0:K9].reshape(1, K9 * K9), (128, 1))
    tstart9 = np.tile(tr[0:K9, START][None, :], (128, 1))
    tstop9 = np.tile(tr[STOP, 0:K9][None, :], (BL, 1))
    wvec9 = np.tile((K9 - 1 - np.arange(K9, dtype=f32))[None, :], (128, 1))
    ivec9 = np.tile(np.arange(K9, dtype=f32)[None, :], (128, 1))
    ident = np.eye(128, dtype=f32)
    embed = np.asarray(embed, f32)
    sentence = np.asarray(sentence)

    maps = []
    for core in range(NCORES):
        sl = sentence[core * BL:(core + 1) * BL, :T].astype(np.int32)
        idx_tm = sl.T.reshape(-1)                       # n = t*BL+b
        idx_in = idx_tm.reshape(-1, 128).T.copy()       # [128, NTILE]
        h_i = 2.0 * np.asarray(h0, f32)[:, core * BL:(core + 1) * BL, :]
        c_i = 2.0 * np.asarray(c0, f32)[:, core * BL:(core + 1) * BL, :]
        maps.append({
            "idx_in": idx_in,
            "embed": embed,
            "w_ihT": w_ihT,
            "w_hhT": w_hhT,
            "b_in": b_in,
            "h_init": np.ascontiguousarray(h_i.transpose(0, 2, 1)),
            "c_init": np.ascontiguousarray(c_i.transpose(0, 2, 1)),
            "w_outT": w_outT,
            "bout_rep": bout_rep,
            "ident": ident,
            "tr9": tr9,
            "tstart9": tstart9,
            "tstop9": tstop9,
            "wvec9": wvec9,
            "ivec9": ivec9,
        })
    return maps


_NC_CACHE = {}


def kernel(sentence, h0, c0, embed, W_ih_f, W_hh_f, b_f, W_ih_r, W_hh_r, b_r,
           W_out, b_out, transitions):
    T = np.asarray(sentence).shape[1]
    if T not in _NC_CACHE:
        _NC_CACHE[T] = build_program(T)
    nc = _NC_CACHE[T]
    maps = prep_inputs(sentence, h0, c0, embed, W_ih_f, W_hh_f, b_f,
                       W_ih_r, W_hh_r, b_r, W_out, b_out, transitions, T=T)
    res = run_bass_kernel_spmd(nc, maps, list(range(NCORES)))
    out = np.concatenate([res.results[i]["path_out"] for i in range(NCORES)], axis=0)
    return out.astype(np.int32)
